# revision 27
# baseline (speedup 1.0000x reference)
"""Trainium2 Bass kernel for nn_CarbonGNN (3x SAGEConv + edge MLP + node classifier).

Strategy (dst-sharded graph-parallel across 8 NeuronCores):
  - Nodes are partitioned across the 8 cores by destination ownership
    (6250 nodes/core); every edge lives on the core owning its dst.
  - Node features are kept replicated in a global table [8B, F] rebuilt each
    layer by AllGather; per-edge rows are fetched with dma_gather (int16
    indices). Since the table exceeds 32768 rows, each node's incoming edges
    are split by source window (w0 = table rows < 32768, w1 = the rest) and
    aggregated in two passes:
      pass B (w1): fixed-degree-class gathers -> strided reduce ->
                   dma_scatter_add into a zeroed DRAM accumulator,
      pass A (w0): class gathers -> reduce -> + aggB -> *1/deg -> PE
                   transpose -> mean@Wl + h@Wr + b (+ReLU).
    The canonical slot order of each core is its w0 class sort, so pass A
    needs no scatter.
  - Degree classes: deg_w padded to bucket sizes so the segment-mean is a
    fixed-stride vector reduction; gather lists are laid out so node s of a
    128-slot chunk keeps its P entries on partition s.
  - Edge head: per edge, gather emb[src] (global table, window-split) and
    emb[dst] (local table), PE-transpose into feature-major, K-split
    matmul + ReLU + [hid,1] matmul.
All sharding/index tables are built host-side in numpy; outputs are gathered
and inverse-permuted back to reference order.
"""

import os
import sys
import numpy as np

sys.path.insert(0, "/opt/trn_rl_repo")

_LAST_RESULTS = None
_LAST_RESULTS2 = None  # BassKernelResults of the most recent run (for test harness)

N_NODES = 50000
N_EDGES = 800000
IN_DIM, HID, OUT = 128, 128, 64
NCORES = 8

# Degree-bucket boundaries (P values); deg padded up to the smallest P >= deg.
P_BUCKETS = [4, 8, 12, 16, 20, 24, 28, 32, 40, 48, 64, 96, 128, 192, 256]

DEBUG_PHASE = os.environ.get("KERNEL_PHASE", "full")  # l0|sage|noedge|full
EDGE_CHUNK = 512   # edges per edge-head matmul chunk
EDGE_SUP = 2048    # edges per edge-head gather superchunk
W0 = 32768         # int16 window size


class Plan:
    pass


def _pad_up(x, m):
    return (x + m - 1) // m * m


def _bucket(deg):
    pb = np.asarray(P_BUCKETS, dtype=np.int64)
    bi = np.searchsorted(pb, np.maximum(deg, 1), side="left")
    assert bi.max() < len(pb), f"degree {deg.max()} exceeds largest bucket"
    return bi, pb


def _wrap16(v):
    """int16 idx vector -> [128, n/16] wrapped + replicated tile data."""
    v = np.asarray(v, dtype=np.int16)
    n = v.shape[0]
    assert n % 16 == 0
    return np.tile(v.reshape(n // 16, 16).T, (8, 1)).copy()


class WindowPlan:
    """Per-window class/chunk structure and gather tables."""
    pass


def _window_classes(deg_w, node_core, per_core):
    """deg_w: [N] per-node degree restricted to this window.
    Returns classes [(P, maxcnt)], per-core node order grouped by class."""
    bi, pb = _bucket(deg_w)
    cnt = np.zeros((NCORES, len(pb)), dtype=np.int64)
    np.add.at(cnt, (node_core, bi), 1)
    mx = cnt.max(axis=0)
    used = np.nonzero(mx > 0)[0]
    classes = [(int(pb[i]), int(mx[i])) for i in used]
    order = []  # per core: list over classes of node arrays
    for c in range(NCORES):
        lo, hi = c * per_core, (c + 1) * per_core
        per_class = []
        for i in used:
            per_class.append(np.nonzero(bi[lo:hi] == i)[0] + lo)
        order.append(per_class)
    return classes, order


def build_plan(edge_index: np.ndarray) -> Plan:
    p = Plan()
    src = np.asarray(edge_index[0], dtype=np.int64)
    dst = np.asarray(edge_index[1], dtype=np.int64)
    per_core = N_NODES // NCORES
    node_core = (np.arange(N_NODES) // per_core).astype(np.int64)
    owner = node_core[dst]

    deg = np.bincount(dst, minlength=N_NODES).astype(np.int64)

    # ---- canonical slot layout: classes of TOTAL w0-degree (needs gslot,
    # which needs the layout...). Break the cycle: slot layout from w0-degree
    # computed via a provisional gslot based on plain node order is wrong;
    # instead note window membership only depends on gslot < W0, i.e. on the
    # slot layout itself. Use a fixed-point-free approach: lay out slots by
    # TOTAL degree class (independent of windows); windows then split each
    # node's edges by src gslot.
    bi, pb = _bucket(deg)
    cnt = np.zeros((NCORES, len(pb)), dtype=np.int64)
    np.add.at(cnt, (node_core, bi), 1)
    mx = cnt.max(axis=0)
    used = np.nonzero(mx > 0)[0]
    classesT = [(int(pb[i]), int(mx[i])) for i in used]
    class_start = []
    s = 0
    for P, c in classesT:
        class_start.append(s)
        s += _pad_up(c, 128)
    S = s
    B = S + 1
    p.S, p.B = S, B

    slot_of = np.full(N_NODES, -1, dtype=np.int64)
    node_at = np.full((NCORES, S), -1, dtype=np.int64)
    for c in range(NCORES):
        lo, hi = c * per_core, (c + 1) * per_core
        for k, i in enumerate(used):
            nodes = np.nonzero(bi[lo:hi] == i)[0] + lo
            s0 = class_start[k]
            slot_of[nodes] = s0 + np.arange(len(nodes))
            node_at[c, s0:s0 + len(nodes)] = nodes
    p.slot_of = slot_of
    p.node_at = node_at
    gslot = node_core * B + slot_of
    p.gslot = gslot

    # universal in-window zero rows (core0 / core7 zero rows)
    z0 = 0 * B + S           # < W0
    z1 = 7 * B + S           # >= W0
    assert z0 < W0 <= z1 < NCORES * B
    p.z0, p.z1 = z0, z1

    # ---- per-window per-node edge lists ----
    order = np.argsort(dst, kind="stable")
    src_s, dst_s = src[order], dst[order]
    edge_start = np.zeros(N_NODES + 1, dtype=np.int64)
    np.cumsum(deg, out=edge_start[1:])
    src_gs = gslot[src_s]          # global slot of each (dst-sorted) edge src
    in_w0 = src_gs < W0

    deg_w = np.zeros((2, N_NODES), dtype=np.int64)
    np.add.at(deg_w[0], dst_s[in_w0], 1)
    np.add.at(deg_w[1], dst_s[~in_w0], 1)

    p.win = []
    for w in (0, 1):
        wp = WindowPlan()
        classes, orderw = _window_classes(deg_w[w], node_core, per_core)
        # chunk list (P, n_real) in class order, chunks of up to 128 nodes
        chunks = []
        for k, (P, cmax) in enumerate(classes):
            nch = _pad_up(cmax, 128) // 128
            for j in range(nch):
                chunks.append((P, min(128, cmax - j * 128)))
        wp.classes = classes
        wp.chunks = chunks
        G = sum(128 * P for (P, _n) in chunks)   # gather idx count (full 128)
        wp.G = G

        zrow = p.z0 if w == 0 else p.z1
        base = 0 if w == 0 else W0
        nch_total = len(chunks)
        # chunk flat-position bases within the G-long gather list
        chunk_base = np.zeros(nch_total, dtype=np.int64)
        acc = 0
        for ci, (P, _n) in enumerate(chunks):
            chunk_base[ci] = acc
            acc += 128 * P
        assert acc == G
        # per class: index of its first chunk
        class_chunk0 = np.zeros(len(classes), dtype=np.int64)
        a = 0
        for k, (P, cmax) in enumerate(classes):
            class_chunk0[k] = a
            a += _pad_up(cmax, 128) // 128

        wsel = in_w0 if w == 0 else ~in_w0
        cmask = np.cumsum(wsel)
        P0 = np.concatenate([[0], cmask])  # prefix count of window edges
        # rank of each window edge within its node's window list
        epos_g = np.nonzero(wsel)[0]
        rank = cmask[epos_g] - 1 - P0[edge_start[dst_s[epos_g]]]

        # per-node class position j_n (within its core's class order)
        bi_w, _pb = _bucket(deg_w[w])
        # order nodes by (core, class bucket, node id) and enumerate per group
        okey = np.lexsort((np.arange(N_NODES), bi_w, node_core))
        grp = node_core[okey] * len(_pb) + bi_w[okey]
        newgrp = np.concatenate([[True], grp[1:] != grp[:-1]])
        gstart = np.zeros(len(grp), dtype=np.int64)
        gstart[newgrp] = np.arange(len(grp))[newgrp]
        gstart = np.maximum.accumulate(gstart)
        j_in_class = np.arange(len(grp)) - gstart
        j_n = np.empty(N_NODES, dtype=np.int64)
        j_n[okey] = j_in_class
        # map bucket index -> class index k (classes are in bucket order)
        b2k = -np.ones(len(_pb), dtype=np.int64)
        for k, (Pv, _m) in enumerate(classes):
            b2k[int(np.searchsorted(_pb, Pv))] = k

        gidx = np.full((NCORES, G), zrow - base, dtype=np.int16)
        # flat position for each window edge
        n_of_e = dst_s[epos_g]
        k_of_e = b2k[bi_w[n_of_e]]
        ch_of_e = class_chunk0[k_of_e] + j_n[n_of_e] // 128
        s_of_e = j_n[n_of_e] % 128
        flatpos = chunk_base[ch_of_e] + rank * 128 + s_of_e
        core_of_e = node_core[n_of_e]
        vals = (src_gs[epos_g] - base).astype(np.int16)
        gidx[core_of_e, flatpos] = vals

        # scatter slot ids per chunk row (trash rows S+s for dummies)
        sidx = np.tile(np.arange(128, dtype=np.int16) + S,
                       (NCORES, nch_total, 1))
        ch_of_n = class_chunk0[b2k[bi_w]] + j_n // 128
        sidx[node_core, ch_of_n, j_n % 128] = slot_of.astype(np.int16)
        wp.gidx = gidx          # values already window-local
        wp.sidx = sidx
        wp.nodes_per_chunk = [n for (_P, n) in chunks]
        p.win.append(wp)

    # canonical order for window 0 must equal the slot layout. Verify:
    # window-0 class of node == total class iff deg buckets coincide, which
    # they don't in general — so pass A (w0) ALSO scatters? No: we instead
    # let BOTH windows scatter-add into the accumulator and run the
    # mean/matmul phase over canonical chunks reading it. (Simpler & uniform.)

    # inv_cnt in canonical slot layout, [128, S/128]
    inv_sb = np.zeros((NCORES, 128, S // 128), dtype=np.float32)
    for c in range(NCORES):
        nn = p.node_at[c]
        iv = np.where(nn >= 0, 1.0 / np.maximum(deg[np.maximum(nn, 0)], 1), 0.0)
        inv_sb[c] = iv.reshape(S // 128, 128).T.astype(np.float32)
    p.inv_sb = inv_sb

    # ---- edge head tables ----
    ecnt0 = np.zeros(NCORES, dtype=np.int64)
    ecnt1 = np.zeros(NCORES, dtype=np.int64)
    owner_s = owner[order]
    for c in range(NCORES):
        sel = owner_s == c
        ecnt0[c] = int((sel & in_w0).sum())
        ecnt1[c] = int((sel & ~in_w0).sum())
    EC0 = _pad_up(max(int(ecnt0.max()), 1), EDGE_SUP)
    EC1 = _pad_up(max(int(ecnt1.max()), 1), EDGE_SUP)
    p.EC0, p.EC1 = EC0, EC1
    EC = EC0 + EC1
    p.EC = EC
    esrc = np.zeros((NCORES, EC), dtype=np.int16)
    edst = np.zeros((NCORES, EC), dtype=np.int16)   # local slot ids
    epos = []
    for c in range(NCORES):
        sel0 = np.nonzero((owner_s == c) & in_w0)[0]
        sel1 = np.nonzero((owner_s == c) & ~in_w0)[0]
        s0v = np.full(EC0, p.z0, dtype=np.int64)
        s0v[:len(sel0)] = src_gs[sel0]
        s1v = np.full(EC1, p.z1 - W0, dtype=np.int64)
        s1v[:len(sel1)] = src_gs[sel1] - W0
        esrc[c] = np.concatenate([s0v, s1v]).astype(np.int16)
        d0 = np.full(EC0, S, dtype=np.int64)
        d0[:len(sel0)] = slot_of[dst_s[sel0]]
        d1 = np.full(EC1, S, dtype=np.int64)
        d1[:len(sel1)] = slot_of[dst_s[sel1]]
        edst[c] = np.concatenate([d0, d1]).astype(np.int16)
        epos.append((order[sel0], order[sel1]))
    p.esrc = esrc
    p.edst = edst
    p.epos = epos
    return p


def _shard_x(p: Plan, x: np.ndarray) -> np.ndarray:
    xs = np.zeros((NCORES, p.B, IN_DIM), dtype=np.float32)
    for c in range(NCORES):
        m = p.node_at[c] >= 0
        xs[c, :p.S][m] = x[p.node_at[c][m]]
    return xs


def golden(p, x, Ws, heads):
    """Numpy emulation of the device dataflow."""
    xs = _shard_x(p, x)
    S, B = p.S, p.B
    tab = np.concatenate([xs[c] for c in range(NCORES)], axis=0)
    own = [xs[c] for c in range(NCORES)]

    for li, (Wl, Wr, b) in enumerate(Ws):
        H = Wl.shape[1]
        new_own = [np.zeros((B, H), np.float32) for _ in range(NCORES)]
        for c in range(NCORES):
            agg = np.zeros((S + 128, tab.shape[1]), np.float32)  # + trash rows
            for w in (0, 1):
                wp = p.win[w]
                base = 0 if w == 0 else W0
                pos = 0
                for ci, (P, n_real) in enumerate(wp.chunks):
                    idx = wp.gidx[c][pos:pos + 128 * P].astype(np.int64) + base
                    pos += 128 * P
                    g = tab[idx].reshape(P, 128, -1)  # position i = e*128+s
                    red = g.sum(axis=0)               # [128, F]
                    sl = wp.sidx[c, ci].astype(np.int64)
                    agg[sl] += red[:128]
            # inv_sb[p, j] corresponds to slot j*128 + p
            inv_flat = p.inv_sb[c].T.ravel()
            mean = agg[:S] * inv_flat[:, None]
            hcur = own[c][:S]
            o = mean @ Wl + hcur @ Wr + b
            if li < 2:
                o = np.maximum(o, 0.0)
            new_own[c][:S] = o
        own = new_own
        tab = np.concatenate([own[c] for c in range(NCORES)], axis=0)

    Wf1, bf1, Wf2, bf2, Wc, bc = heads
    flows_c, sup_c = [], []
    for c in range(NCORES):
        e0 = tab[p.esrc[c][:p.EC0].astype(np.int64)]
        e1 = tab[p.esrc[c][p.EC0:].astype(np.int64) + W0]
        es = np.concatenate([e0, e1], axis=0)
        ed = own[c][p.edst[c].astype(np.int64)]
        ee = np.concatenate([es, ed], axis=1)
        f = np.maximum(ee @ Wf1 + bf1, 0.0) @ Wf2 + bf2
        flows_c.append(f)
        sup_c.append(own[c][:S] @ Wc + bc)

    node_emb = np.zeros((N_NODES, OUT), np.float32)
    sup = np.zeros((N_NODES, 4), np.float32)
    for c in range(NCORES):
        m = p.node_at[c] >= 0
        node_emb[p.node_at[c][m]] = own[c][:S][m]
        sup[p.node_at[c][m]] = sup_c[c][:S][m]
    flows = np.zeros((N_EDGES, 1), np.float32)
    for c in range(NCORES):
        w0pos, w1pos = p.epos[c]
        flows[w0pos] = flows_c[c][:len(w0pos)]
        flows[w1pos] = flows_c[c][p.EC0:p.EC0 + len(w1pos)]
    return node_emb, flows, sup


# ---------------------------------------------------------------------------
# Bass program
# ---------------------------------------------------------------------------


def build_bass(p: Plan):
    import concourse.bass as bass
    import concourse.mybir as mybir
    import concourse.bacc as bacc
    import concourse.tile as tile

    f32 = mybir.dt.float32
    i16 = mybir.dt.int16
    AF = mybir.ActivationFunctionType
    S, B, EC = p.S, p.B, p.EC
    NCOLS = S // 128

    nc = bacc.Bacc(num_devices=NCORES)

    # ---- I/O ----
    x_own = nc.declare_dram_parameter("x_own", [B, IN_DIM], f32, isOutput=False)
    GA, GB = p.win[0].G, p.win[1].G
    gidx0 = nc.declare_dram_parameter("gidx0", [128, GA // 16], i16, isOutput=False)
    gidx1 = nc.declare_dram_parameter("gidx1", [128, GB // 16], i16, isOutput=False)
    nch0, nch1 = len(p.win[0].chunks), len(p.win[1].chunks)
    sidx0 = nc.declare_dram_parameter("sidx0", [128, nch0 * 8], i16, isOutput=False)
    sidx1 = nc.declare_dram_parameter("sidx1", [128, nch1 * 8], i16, isOutput=False)
    esrc = nc.declare_dram_parameter("esrc", [128, EC // 16], i16, isOutput=False)
    edst = nc.declare_dram_parameter("edst", [128, EC // 16], i16, isOutput=False)
    inv_in = nc.declare_dram_parameter("inv_sb", [128, NCOLS], f32, isOutput=False)
    wnames = {}
    for nm, shape in [
        ("Wl0", [IN_DIM, HID]), ("Wr0", [IN_DIM, HID]), ("b0", [1, HID]),
        ("Wl1", [HID, HID]), ("Wr1", [HID, HID]), ("b1", [1, HID]),
        ("Wl2", [HID, OUT]), ("Wr2", [HID, OUT]), ("b2", [1, OUT]),
        ("Wf1", [2 * OUT, HID]), ("bf1", [HID, 1]), ("Wf2", [HID, 1]),
        ("bf2", [1, 1]), ("Wc", [OUT, 4]), ("bc", [1, 4]),
        ("ident", [128, 128]),
        ("ones", [1, 128]),
    ]:
        wnames[nm] = nc.declare_dram_parameter(nm, shape, f32, isOutput=False)
    emb_own = nc.declare_dram_parameter("emb_own", [S, OUT], f32, isOutput=True)
    sup_own = nc.declare_dram_parameter("sup_own", [S, 4], f32, isOutput=True)
    flows_own = nc.declare_dram_parameter("flows_own", [1, EC], f32, isOutput=True)

    # ---- internal DRAM ----
    agin128 = nc.dram_tensor("agin128", [B, 128], f32)
    agout128 = nc.dram_tensor("agout128", [NCORES * B, 128], f32,
                              addr_space="Shared")
    agin64 = nc.dram_tensor("agin64", [B, OUT], f32)
    agout64 = nc.dram_tensor("agout64", [NCORES * B, OUT], f32,
                             addr_space="Shared")
    aggB = nc.dram_tensor("aggB", [S + 128, 128], f32)
    zeros_d = nc.inline_tensor(np.zeros((S + 128, 128), np.float32), "zeros_d")
    rg = [list(range(NCORES))]

    max_p0 = max(P for (P, _n) in p.win[0].chunks)
    max_p1 = max(P for (P, _n) in p.win[1].chunks)
    max_p = max(max_p0, max_p1)

    with tile.TileContext(nc) as tc:
        with (
            tc.tile_pool(name="const", bufs=1) as cpool,
            tc.tile_pool(name="ht", bufs=1) as hpool,
            tc.tile_pool(name="gt", bufs=2) as gpool,
            tc.tile_pool(name="wk", bufs=3) as wpool,
            tc.tile_pool(name="ps", bufs=2, space="PSUM") as ppool,
        ):
            # ---- constants ----
            ct = {}
            for nm, t in wnames.items():
                sb = cpool.tile(list(t.shape), f32, tag=f"c_{nm}")
                nc.sync.dma_start(out=sb[:], in_=t[:])
                ct[nm] = sb
            inv_t = cpool.tile([128, NCOLS], f32, tag="c_inv")
            nc.sync.dma_start(out=inv_t[:], in_=inv_in[:])
            zrow = cpool.tile([1, 128], f32, tag="c_zrow")
            nc.vector.memset(zrow[:], 0.0)
            # Wf1 halves as separate tiles (matmul needs base partition 0)
            wf1a = cpool.tile([OUT, HID], f32, tag="c_wf1a")
            nc.sync.dma_start(out=wf1a[:], in_=wnames["Wf1"][0:OUT, :])
            wf1b = cpool.tile([OUT, HID], f32, tag="c_wf1b")
            nc.sync.dma_start(out=wf1b[:], in_=wnames["Wf1"][OUT:2 * OUT, :])

            # index preloads (reused across layers)
            gi0 = cpool.tile([128, GA // 16], i16, tag="c_gi0")
            nc.sync.dma_start(out=gi0[:], in_=gidx0[:])
            gi1 = cpool.tile([128, GB // 16], i16, tag="c_gi1")
            nc.sync.dma_start(out=gi1[:], in_=gidx1[:])
            si0 = cpool.tile([128, nch0 * 8], i16, tag="c_si0")
            nc.sync.dma_start(out=si0[:], in_=sidx0[:])
            si1 = cpool.tile([128, nch1 * 8], i16, tag="c_si1")
            nc.sync.dma_start(out=si1[:], in_=sidx1[:])

            # x -> agin128, zero-row of agin64, AllGather x table
            nc.sync.dma_start(out=agin128[:, :], in_=x_own[:, :])
            nc.sync.dma_start(out=agin64[S:S + 1, :], in_=zrow[0:1, :OUT])
            nc.gpsimd.collective_compute(
                "AllGather", mybir.AluOpType.bypass, replica_groups=rg,
                ins=[agin128[:].opt()], outs=[agout128[:].opt()])

            # persistent transposed activations [feat, slot]
            hT_a = hpool.tile([128, S], f32, tag="hA")   # xT, then h2T
            hT_b = hpool.tile([128, S], f32, tag="hB")   # h1T

            for s0 in range(0, S, 128):
                xr = wpool.tile([128, 128], f32, tag="hrow")
                nc.sync.dma_start(out=xr[:], in_=x_own[s0:s0 + 128, :])
                pt = ppool.tile([128, 512], f32, tag="tp")
                nc.tensor.transpose(pt[:, :128], xr[:], ct["ident"][:])
                nc.vector.tensor_copy(hT_a[:, s0:s0 + 128], pt[:, :128])

            layer_cfg = [
                (ct["Wl0"], ct["Wr0"], ct["b0"], hT_a, hT_b, 128, True),
                (ct["Wl1"], ct["Wr1"], ct["b1"], hT_b, hT_a, 128, True),
                (ct["Wl2"], ct["Wr2"], ct["b2"], hT_a, hT_b, OUT, False),
            ]
            if DEBUG_PHASE == "l0":
                layer_cfg = layer_cfg[:1]

            def win_pass(w, htab):
                """Gather+reduce+scatter-add one window into aggB."""
                wp = p.win[w]
                gi = gi0 if w == 0 else gi1
                si = si0 if w == 0 else si1
                base = 0 if w == 0 else W0
                hi = W0 if w == 0 else NCORES * B
                tabv = htab[base:hi, :]
                pos = 0
                for ci, (P, n_real) in enumerate(wp.chunks):
                    n_idx = 128 * P
                    gt = gpool.tile([128, max_p * 128], f32, tag="gt")
                    nc.gpsimd.dma_gather(
                        out_ap=gt[:, :P * 128].rearrange(
                            "p (e f) -> p e f", f=128),
                        in_ap=tabv,
                        idxs_ap=gi[:, pos // 16:(pos + n_idx) // 16],
                        num_idxs=n_idx, num_idxs_reg=n_idx,
                        elem_size=128, single_packet=False)
                    red = wpool.tile([128, 128], f32, tag="red")
                    nc.vector.tensor_reduce(
                        out=red[:],
                        in_=gt[:, :P * 128].rearrange(
                            "p (e f) -> p f e", f=128),
                        axis=mybir.AxisListType.X, op=mybir.AluOpType.add)
                    nc.gpsimd.dma_scatter_add(
                        out_ap=aggB[:, :],
                        in_ap=red[:].rearrange("p (e f) -> p e f", f=128),
                        idxs_ap=si[:, ci * 8:(ci + 1) * 8],
                        num_idxs=128, num_idxs_reg=128,
                        elem_size=128, single_packet=False)
                    pos += n_idx
                assert pos == wp.G

            for li, (Wl, Wr, b, hT_in, hT_out, H, do_relu) in enumerate(layer_cfg):
                agin = agin128 if li < 2 else agin64
                # zero the accumulator, then scatter both windows into it
                nc.sync.dma_start(out=aggB[:, :], in_=zeros_d[:, :])
                win_pass(1, agout128)
                win_pass(0, agout128)
                # mean/matmul phase over canonical chunks
                for j in range(NCOLS):
                    s0 = j * 128
                    agg = wpool.tile([128, 128], f32, tag="agg")
                    nc.sync.dma_start(out=agg[:], in_=aggB[s0:s0 + 128, :])
                    mean = wpool.tile([128, 128], f32, tag="mean")
                    nc.vector.tensor_scalar_mul(
                        mean[:], agg[:], inv_t[:, j:j + 1])
                    pt = ppool.tile([128, 512], f32, tag="tp")
                    nc.tensor.transpose(pt[:, :128], mean[:], ct["ident"][:])
                    meanT = wpool.tile([128, 128], f32, tag="meanT")
                    nc.vector.tensor_copy(meanT[:], pt[:, :128])
                    pm = ppool.tile([128, 512], f32, tag="mm")
                    nc.tensor.matmul(pm[:, :H], meanT[:], Wl[:],
                                     start=True, stop=False)
                    nc.tensor.matmul(pm[:, :H], hT_in[:, s0:s0 + 128], Wr[:],
                                     start=False, stop=False)
                    nc.tensor.matmul(pm[:, :H], ct["ones"][:], b[:],
                                     start=False, stop=True)
                    hrow = wpool.tile([128, 128], f32, tag="hrow")
                    nc.scalar.activation(hrow[:, :H], pm[:, :H],
                                         AF.Relu if do_relu else AF.Copy)
                    nc.scalar.dma_start(out=agin[s0:s0 + 128, :],
                                        in_=hrow[:, :H])
                    pt2 = ppool.tile([128, 512], f32, tag="tp")
                    nc.tensor.transpose(pt2[0:H, 0:128], hrow[:, :H],
                                        ct["ident"][:])
                    nc.vector.tensor_copy(hT_out[0:H, s0:s0 + 128],
                                          pt2[0:H, :128])
                tc.strict_bb_all_engine_barrier()
                if li < 2:
                    nc.gpsimd.collective_compute(
                        "AllGather", mybir.AluOpType.bypass, replica_groups=rg,
                        ins=[agin128[:].opt()], outs=[agout128[:].opt()])
                else:
                    nc.gpsimd.collective_compute(
                        "AllGather", mybir.AluOpType.bypass, replica_groups=rg,
                        ins=[agin64[:].opt()], outs=[agout64[:].opt()])

            if DEBUG_PHASE == "l0":
                nc.sync.dma_start(out=emb_own[:, :OUT],
                                  in_=agin128[0:S, :OUT])
            else:
                nc.sync.dma_start(out=emb_own[:, :], in_=agin64[0:S, :])

            # ---- supplier classifier ----
            run_heads = DEBUG_PHASE in ("full", "noedge")
            embT = hT_b
            for j in range(NCOLS if run_heads else 0):
                s0 = j * 128
                pm = ppool.tile([128, 512], f32, tag="mm")
                nc.tensor.matmul(pm[:, :4], embT[0:OUT, s0:s0 + 128],
                                 ct["Wc"][:], start=True, stop=False)
                nc.tensor.matmul(pm[:, :4], ct["ones"][:], ct["bc"][:],
                                 start=False, stop=True)
                st = wpool.tile([128, 4], f32, tag="sup")
                nc.vector.tensor_copy(st[:], pm[:, :4])
                nc.scalar.dma_start(out=sup_own[s0:s0 + 128, :], in_=st[:])

            # ---- edge head ----
            _elim = int(os.environ.get("KERNEL_EDGE_LIMIT", "0")) or EC
            for e0 in range(0, min(EC, _elim) if DEBUG_PHASE == "full" else 0,
                            EDGE_SUP):
                ns = min(EDGE_SUP, EC - e0)
                nblk = ns // 128
                if e0 < p.EC0:
                    srctab = agout64[0:W0, :]
                else:
                    srctab = agout64[W0:NCORES * B, :]
                eis = wpool.tile([128, EDGE_SUP // 16], i16, tag="eis", bufs=2)
                eid = wpool.tile([128, EDGE_SUP // 16], i16, tag="eid", bufs=2)
                nc.sync.dma_start(out=eis[:, :ns // 16],
                                  in_=esrc[:, e0 // 16:(e0 + ns) // 16])
                nc.sync.dma_start(out=eid[:, :ns // 16],
                                  in_=edst[:, e0 // 16:(e0 + ns) // 16])
                gs = gpool.tile([128, (EDGE_SUP // 128) * OUT], f32, tag="eg_s")
                gd = gpool.tile([128, (EDGE_SUP // 128) * OUT], f32, tag="eg_d")
                nc.gpsimd.dma_gather(
                    out_ap=gs[:, :nblk * OUT].rearrange(
                        "p (e f) -> p e f", f=OUT),
                    in_ap=srctab,
                    idxs_ap=eis[:, :ns // 16],
                    num_idxs=ns, num_idxs_reg=ns,
                    elem_size=OUT, single_packet=False)
                if os.environ.get("KERNEL_EDGE_NODST"):
                    nc.vector.memset(gd[:], 0.0)
                else:
                    nc.gpsimd.dma_gather(
                        out_ap=gd[:, :nblk * OUT].rearrange(
                            "p (e f) -> p e f", f=OUT),
                        in_ap=agin64[:, :],
                        idxs_ap=eid[:, :ns // 16],
                        num_idxs=ns, num_idxs_reg=ns,
                        elem_size=OUT, single_packet=False)
                fl = wpool.tile([1, EDGE_SUP], f32, tag="fl", bufs=2)
                for sub in range(ns // EDGE_CHUNK):
                    peeS = ppool.tile([64, 512], f32, tag="tp")
                    peeD = ppool.tile([64, 512], f32, tag="tpD")
                    for jj in range(4):
                        blk = sub * 4 + jj
                        nc.tensor.transpose(
                            peeS[:, jj * 128:(jj + 1) * 128],
                            gs[:, blk * OUT:(blk + 1) * OUT], ct["ident"][:])
                        nc.tensor.transpose(
                            peeD[:, jj * 128:(jj + 1) * 128],
                            gd[:, blk * OUT:(blk + 1) * OUT], ct["ident"][:])
                    eeS = wpool.tile([64, 512], f32, tag="eeS", bufs=2)
                    eeD = wpool.tile([64, 512], f32, tag="eeD", bufs=2)
                    nc.vector.tensor_copy(eeS[:], peeS[:])
                    nc.vector.tensor_copy(eeD[:], peeD[:])
                    pm1 = ppool.tile([128, 512], f32, tag="mm")
                    nc.tensor.matmul(pm1[:], wf1a[:], eeS[:],
                                     start=True, stop=False)
                    nc.tensor.matmul(pm1[:], wf1b[:], eeD[:],
                                     start=False, stop=True)
                    r1 = wpool.tile([128, 512], f32, tag="r1", bufs=2)
                    nc.scalar.activation(r1[:], pm1[:], AF.Relu,
                                         bias=ct["bf1"][:, 0:1])
                    po = ppool.tile([1, 512], f32, tag="o2")
                    nc.tensor.matmul(po[:], ct["Wf2"][:], r1[:],
                                     start=True, stop=True)
                    nc.vector.tensor_scalar_add(
                        fl[0:1, sub * 512:(sub + 1) * 512], po[:],
                        ct["bf2"][0:1, 0:1])
                nc.scalar.dma_start(out=flows_own[0:1, e0:e0 + ns],
                                    in_=fl[0:1, :ns])

    nc.finalize()
    return nc


def build_bass_edge(p: Plan):
    """Standalone edge-head program (second launch)."""
    import concourse.bass as bass
    import concourse.mybir as mybir
    import concourse.bacc as bacc
    import concourse.tile as tile

    f32 = mybir.dt.float32
    i16 = mybir.dt.int16
    AF = mybir.ActivationFunctionType
    S, B, EC = p.S, p.B, p.EC

    nc = bacc.Bacc(num_devices=NCORES)
    etab_g = nc.declare_dram_parameter("etab_g", [NCORES * B, OUT], f32,
                                       isOutput=False)
    etab_l = nc.declare_dram_parameter("etab_l", [B, OUT], f32, isOutput=False)
    esrc = nc.declare_dram_parameter("esrc", [128, EC // 16], i16, isOutput=False)
    edst = nc.declare_dram_parameter("edst", [128, EC // 16], i16, isOutput=False)
    wn = {}
    for nm, shape in [("Wf1", [2 * OUT, HID]), ("bf1", [HID, 1]),
                      ("Wf2", [HID, 1]), ("bf2", [1, 1]),
                      ("ident", [128, 128])]:
        wn[nm] = nc.declare_dram_parameter(nm, shape, f32, isOutput=False)
    flows_own = nc.declare_dram_parameter("flows_own", [1, EC], f32,
                                          isOutput=True)

    with tile.TileContext(nc) as tc:
        with (
            tc.tile_pool(name="const", bufs=1) as cpool,
            tc.tile_pool(name="gt", bufs=2) as gpool,
            tc.tile_pool(name="wk", bufs=3) as wpool,
            tc.tile_pool(name="ps", bufs=2, space="PSUM") as ppool,
        ):
            ct = {}
            for nm, t in wn.items():
                sb = cpool.tile(list(t.shape), f32, tag=f"c_{nm}")
                nc.sync.dma_start(out=sb[:], in_=t[:])
                ct[nm] = sb
            wf1a = cpool.tile([OUT, HID], f32, tag="c_wf1a")
            nc.sync.dma_start(out=wf1a[:], in_=wn["Wf1"][0:OUT, :])
            wf1b = cpool.tile([OUT, HID], f32, tag="c_wf1b")
            nc.sync.dma_start(out=wf1b[:], in_=wn["Wf1"][OUT:2 * OUT, :])

            for e0 in range(0, EC, EDGE_SUP):
                ns = min(EDGE_SUP, EC - e0)
                nblk = ns // 128
                srctab = etab_g[0:W0, :] if e0 < p.EC0 else \
                    etab_g[W0:NCORES * B, :]
                eis = wpool.tile([128, EDGE_SUP // 16], i16, tag="eis", bufs=2)
                eid = wpool.tile([128, EDGE_SUP // 16], i16, tag="eid", bufs=2)
                nc.sync.dma_start(out=eis[:, :ns // 16],
                                  in_=esrc[:, e0 // 16:(e0 + ns) // 16])
                nc.sync.dma_start(out=eid[:, :ns // 16],
                                  in_=edst[:, e0 // 16:(e0 + ns) // 16])
                gs = gpool.tile([128, (EDGE_SUP // 128) * OUT], f32, tag="eg_s")
                gd = gpool.tile([128, (EDGE_SUP // 128) * OUT], f32, tag="eg_d")
                nc.gpsimd.dma_gather(
                    out_ap=gs[:, :nblk * OUT].rearrange(
                        "p (e f) -> p e f", f=OUT),
                    in_ap=srctab, idxs_ap=eis[:, :ns // 16],
                    num_idxs=ns, num_idxs_reg=ns,
                    elem_size=OUT, single_packet=False)
                nc.gpsimd.dma_gather(
                    out_ap=gd[:, :nblk * OUT].rearrange(
                        "p (e f) -> p e f", f=OUT),
                    in_ap=etab_l[:, :], idxs_ap=eid[:, :ns // 16],
                    num_idxs=ns, num_idxs_reg=ns,
                    elem_size=OUT, single_packet=False)
                fl = wpool.tile([1, EDGE_SUP], f32, tag="fl", bufs=2)
                for sub in range(ns // EDGE_CHUNK):
                    peeS = ppool.tile([64, 512], f32, tag="tp")
                    peeD = ppool.tile([64, 512], f32, tag="tpD")
                    for jj in range(4):
                        blk = sub * 4 + jj
                        nc.tensor.transpose(
                            peeS[:, jj * 128:(jj + 1) * 128],
                            gs[:, blk * OUT:(blk + 1) * OUT], ct["ident"][:])
                        nc.tensor.transpose(
                            peeD[:, jj * 128:(jj + 1) * 128],
                            gd[:, blk * OUT:(blk + 1) * OUT], ct["ident"][:])
                    eeS = wpool.tile([64, 512], f32, tag="eeS", bufs=2)
                    eeD = wpool.tile([64, 512], f32, tag="eeD", bufs=2)
                    nc.vector.tensor_copy(eeS[:], peeS[:])
                    nc.vector.tensor_copy(eeD[:], peeD[:])
                    pm1 = ppool.tile([128, 512], f32, tag="mm")
                    nc.tensor.matmul(pm1[:], wf1a[:], eeS[:],
                                     start=True, stop=False)
                    nc.tensor.matmul(pm1[:], wf1b[:], eeD[:],
                                     start=False, stop=True)
                    r1 = wpool.tile([128, 512], f32, tag="r1", bufs=2)
                    nc.scalar.activation(r1[:], pm1[:], AF.Relu,
                                         bias=ct["bf1"][:, 0:1])
                    po = ppool.tile([1, 512], f32, tag="o2")
                    nc.tensor.matmul(po[:], ct["Wf2"][:], r1[:],
                                     start=True, stop=True)
                    nc.vector.tensor_scalar_add(
                        fl[0:1, sub * 512:(sub + 1) * 512], po[:],
                        ct["bf2"][0:1, 0:1])
                nc.scalar.dma_start(out=flows_own[0:1, e0:e0 + ns],
                                    in_=fl[0:1, :ns])

    nc.finalize()
    return nc


def _make_in_maps(p: Plan, inputs):
    xs = _shard_x(p, np.asarray(inputs["x"], dtype=np.float32))
    ident = np.eye(128, dtype=np.float32)
    ones = np.ones((1, 128), dtype=np.float32)
    in_maps = []
    for c in range(NCORES):
        m = {
            "x_own": xs[c],
            "gidx0": _wrap16(p.win[0].gidx[c]),
            "gidx1": _wrap16(p.win[1].gidx[c]),
            "sidx0": np.concatenate(
                [_wrap16(p.win[0].sidx[c, ci]) for ci in
                 range(len(p.win[0].chunks))], axis=1),
            "sidx1": np.concatenate(
                [_wrap16(p.win[1].sidx[c, ci]) for ci in
                 range(len(p.win[1].chunks))], axis=1),
            "esrc": _wrap16(p.esrc[c]),
            "edst": _wrap16(p.edst[c]),
            "inv_sb": p.inv_sb[c],
            "ident": ident,
            "ones": ones,
        }
        for nm in ["Wl0", "Wr0", "Wl1", "Wr1", "Wl2", "Wr2", "Wf1", "Wf2", "Wc"]:
            m[nm] = np.ascontiguousarray(np.asarray(inputs[nm], np.float32))
        m["b0"] = np.asarray(inputs["b0"], np.float32)[None, :]
        m["b1"] = np.asarray(inputs["b1"], np.float32)[None, :]
        m["b2"] = np.asarray(inputs["b2"], np.float32)[None, :]
        m["bf1"] = np.asarray(inputs["bf1"], np.float32)[:, None]
        m["bf2"] = np.asarray(inputs["bf2"], np.float32)[None, :]
        m["bc"] = np.asarray(inputs["bc"], np.float32)[None, :]
        in_maps.append(m)
    return in_maps


def _assemble(p: Plan, results):
    node_emb = np.zeros((N_NODES, OUT), np.float32)
    sup = np.zeros((N_NODES, 4), np.float32)
    flows = np.zeros((N_EDGES, 1), np.float32)
    for c in range(NCORES):
        m = p.node_at[c] >= 0
        node_emb[p.node_at[c][m]] = results[c]["emb_own"][m]
        sup[p.node_at[c][m]] = results[c]["sup_own"][m]
        w0pos, w1pos = p.epos[c]
        fc = results[c]["flows_own"][0]
        flows[w0pos, 0] = fc[:len(w0pos)]
        flows[w1pos, 0] = fc[p.EC0:p.EC0 + len(w1pos)]
    return node_emb, flows, sup


def kernel(**inputs):
    global _LAST_RESULTS, _LAST_RESULTS2
    from concourse.bass_utils import run_bass_kernel_spmd

    edge_index = np.asarray(inputs["edge_index"])
    p = build_plan(edge_index)
    global DEBUG_PHASE
    one_launch = os.environ.get("KERNEL_ONE_LAUNCH")
    if not one_launch and DEBUG_PHASE == "full":
        DEBUG_PHASE = "noedge"
    nc = build_bass(p)
    in_maps = _make_in_maps(p, inputs)
    res = run_bass_kernel_spmd(
        nc, in_maps, list(range(NCORES)),
        trace=bool(os.environ.get("BASS_TRACE")),
    )
    _LAST_RESULTS = res
    node_emb, flows, sup = _assemble(p, res.results)
    if one_launch or DEBUG_PHASE != "noedge":
        return node_emb, flows, sup

    # second launch: edge head from the emb results
    S, B = p.S, p.B
    etab_g = np.zeros((NCORES * B, OUT), np.float32)
    for c in range(NCORES):
        etab_g[c * B:c * B + S] = res.results[c]["emb_own"]
    nce = build_bass_edge(p)
    in2 = []
    for c in range(NCORES):
        etab_l = np.zeros((B, OUT), np.float32)
        etab_l[:S] = res.results[c]["emb_own"]
        in2.append({
            "etab_g": etab_g, "etab_l": etab_l,
            "esrc": _wrap16(p.esrc[c]), "edst": _wrap16(p.edst[c]),
            "Wf1": np.ascontiguousarray(np.asarray(inputs["Wf1"], np.float32)),
            "bf1": np.asarray(inputs["bf1"], np.float32)[:, None],
            "Wf2": np.ascontiguousarray(np.asarray(inputs["Wf2"], np.float32)),
            "bf2": np.asarray(inputs["bf2"], np.float32)[None, :],
            "ident": np.eye(128, dtype=np.float32),
        })
    res2 = run_bass_kernel_spmd(
        nce, in2, list(range(NCORES)),
        trace=bool(os.environ.get("BASS_TRACE")),
    )
    _LAST_RESULTS2 = res2
    for c in range(NCORES):
        w0pos, w1pos = p.epos[c]
        fc = res2.results[c]["flows_own"][0]
        flows[w0pos, 0] = fc[:len(w0pos)]
        flows[w1pos, 0] = fc[p.EC0:p.EC0 + len(w1pos)]
    return node_emb, flows, sup
